# revision 36
# baseline (speedup 1.0000x reference)
"""Distributed multi-head attention kernel for 8 Trainium2 NeuronCores.

Problem: B=4, N=2048, E=1024, H=16 heads (head_dim 64), QKV + out projections.
Sharding: core c handles batch b=c//2 and head-group g=c%2 (8 heads = D-slice
of 512). QKV projections are column-sharded, the out projection is row-sharded;
the two partial outputs per batch are summed on the host during unshard.

Per-core dataflow (all matmuls bf16 with fp32 PSUM accumulation):
  A) QpT/KpT [512, 2048] and Vp [2048, 512] projections. Host pre-transposes
     q/k/v to [E, N] so the contraction dim lands on SBUF partitions.
  B) S^T[j, i] = Kp_h^T.T @ Qp_h^T per head. K=64, so head pairs are packed
     onto PE row-groups 0-63 / 64-127 (partition-base-derived tile_position).
     Each head's S^T j-tile goes to its own single-bank [128, 512] PSUM tile.
  C) exp split across engines to break the single-ACT bottleneck: head A's
     tile exp's on the scalar engine (LUT exp fused into the PSUM eviction);
     head B's tile exp's on the DVE as a Schraudolph bf16-bit trick
     (round(s*EXP_A + EXP_B) -> int16, bitcast bf16). The Schraudolph noise
     (~1.9% rms on half the P mass) keeps end-to-end rel err ~1.4e-2 < 2e-2.
  D) O^T_aug[65, i] accumulates Vp_aug^T @ expS^T over j-tiles, where Vp is
     augmented with a ones column so row 64 of the product is the softmax
     denominator Z. Normalize: evict O_aug (DVE), reciprocal (DVE),
     partition-broadcast + multiplies on Pool (SBUF-only engine), partition
     shift 0-63 -> 64-127 via DMA; out-project, store out^T bf16 partials.
  The phases stream as one continuous pipeline: B runs a pair of j-tiles
  ahead of C, V/QK projections and the out-projection are spread as PE
  filler so the array never waits on the exp chain.
"""

import sys

import numpy as np


def _ensure_paths():
    try:
        import concourse.bass  # noqa: F401
    except ImportError:
        for p in ("/opt/trn_rl_repo",):
            if p not in sys.path:
                sys.path.insert(0, p)
        import concourse.bass  # noqa: F401


_ensure_paths()

import ml_dtypes  # noqa: E402
import concourse.bass as bass  # noqa: E402
import concourse.bacc as bacc  # noqa: E402
import concourse.mybir as mybir  # noqa: E402
import concourse.tile as tile  # noqa: E402
from concourse.bass_utils import run_bass_kernel_spmd  # noqa: E402

BF16NP = ml_dtypes.bfloat16

B, N, E = 4, 2048, 1024
H, HD = 16, 64
G = 2                 # head-group (tensor-parallel) factor
S = E // G            # 512: per-core slice of the internal dim
HPC = H // G          # 8 heads per core
ET = E // 128         # 8 contraction tiles for the projections
DT = S // 128         # 4 d-tiles per core (= head pairs)
NT = N // 128         # 16 n-tiles
NBLK = N // 512       # 4 n/i blocks
SCALE = 1.0 / float(np.sqrt(HD))
# Schraudolph exp in bf16-bit space: bf16_bits(exp(s*SCALE)) ~= s*EXP_A + EXP_B
# (DVE f32->int16 convert rounds to nearest; B tuned for ~zero mean bias)
EXP_A = 128.0 * 1.4426950408889634 * SCALE
EXP_B = 127.0 * 128.0 - 7.2

last_exec_time_ns = None
last_results = None


def _install_ntff_shim():
    """Register the axon NTFF profile hook bass_utils wants under trace=True."""
    import types

    if "antenv.axon_hooks" in sys.modules:
        return
    mod = types.ModuleType("antenv.axon_hooks")
    _h = [None]
    mod.set_axon_ntff_profile_hook = lambda h: _h.__setitem__(0, h)
    mod.get_axon_ntff_profile_hook = lambda: _h[0]
    try:
        import antenv

        sys.modules["antenv.axon_hooks"] = mod
        antenv.axon_hooks = mod
        from trn_agent_boot.trn_boot import _ntff_profile_via_ctypes

        hook = _ntff_profile_via_ctypes("/opt/axon/libaxon_pjrt.so")
        if hook is not None:
            mod.set_axon_ntff_profile_hook(hook)
    except Exception:
        pass


def _build(has_bq, has_bk, has_bv, has_bo):
    f32 = mybir.dt.float32
    bf16 = mybir.dt.bfloat16
    i16 = mybir.dt.int16
    PSUM = bass.MemorySpace.PSUM

    nc = bacc.Bacc("TRN2", target_bir_lowering=False, debug=False)

    qT_ext = nc.declare_dram_parameter("qT", [E, N], bf16, isOutput=False)
    kT_ext = nc.declare_dram_parameter("kT", [E, N], bf16, isOutput=False)
    vT_ext = nc.declare_dram_parameter("vT", [E, N], bf16, isOutput=False)
    wq_ext = nc.declare_dram_parameter("wq", [E, S], bf16, isOutput=False)
    wk_ext = nc.declare_dram_parameter("wk", [E, S], bf16, isOutput=False)
    wv_ext = nc.declare_dram_parameter("wv", [E, S], bf16, isOutput=False)
    wo_ext = nc.declare_dram_parameter("wo", [S, E], bf16, isOutput=False)
    bq_ext = nc.declare_dram_parameter("bq", [1, S], bf16, isOutput=False) if has_bq else None
    bk_ext = nc.declare_dram_parameter("bk", [1, S], bf16, isOutput=False) if has_bk else None
    bv_ext = nc.declare_dram_parameter("bv", [1, S], bf16, isOutput=False) if has_bv else None
    bo_ext = nc.declare_dram_parameter("bo", [1, E], bf16, isOutput=False) if has_bo else None
    out_ext = nc.declare_dram_parameter("out", [E, N], bf16, isOutput=True)

    with tile.TileContext(nc) as tc:
        with (
            tc.tile_pool(name="const", bufs=1) as cpool,
            tc.tile_pool(name="w", bufs=1) as wpool,
            tc.tile_pool(name="vin", bufs=2) as vipool,
            tc.tile_pool(name="proj", bufs=1) as ppool,
            tc.tile_pool(name="esA", bufs=3) as esapool,
            tc.tile_pool(name="esB", bufs=3) as esbpool,
            tc.tile_pool(name="on", bufs=13) as onpool,
            tc.tile_pool(name="nrm", bufs=4) as nrmpool,
            tc.tile_pool(name="nrm1", bufs=2) as nrm1pool,
            tc.tile_pool(name="zb", bufs=2) as zbpool,
            tc.tile_pool(name="tmpb", bufs=2) as tbpool,
            tc.tile_pool(name="dst", bufs=4) as dstpool,
            tc.tile_pool(name="ps_s", bufs=5, space=PSUM) as ps_s,
            tc.tile_pool(name="ps_o", bufs=2, space=PSUM) as ps_o,
            tc.tile_pool(name="ps_m", bufs=1, space=PSUM) as ps_m,
        ):
            # ---- constants -------------------------------------------------
            ones_bf = cpool.tile([1, 512], bf16, tag="ones_bf")
            nc.gpsimd.memset(ones_bf[:], 1.0)

            # ---- persistent activations -----------------------------------
            qpT = ppool.tile([128, DT, N], bf16, tag="qpT")   # [d, n], d-tiles = head pairs
            kpT = ppool.tile([128, DT, N], bf16, tag="kpT")
            vpa = ppool.tile([128, NT, HPC * 65], bf16, tag="vpa")  # per head: 64 V cols + ones col
            nc.gpsimd.memset(vpa[:], 1.0)  # pre-set so the ones columns survive the V copies

            # ---- weights / inputs: declared here, DMA'd in priority order --
            wq_t = wpool.tile([128, ET, S], bf16, tag="wq")
            wk_t = wpool.tile([128, ET, S], bf16, tag="wk")
            wv_t = wpool.tile([128, ET, S], bf16, tag="wv")
            wo_t = wpool.tile([128, DT, E], bf16, tag="wo")

            bias_tiles = {}

            # SP issues each DMA descriptor serially (~650ns); the prelude
            # loads alternate between the two HWDGE issuers (SP + ACT) to
            # halve the serial issue time while ACT is still idle.
            _issuer = [0]

            def dma2(out, in_):
                # alternate issuers for the first 24 loads only: past that the
                # ACT queue must stay clear for projection evictions / exps
                # (a queued DMA's queue-credit wait would block them)
                eng = nc.sync if (_issuer[0] % 2 == 0 or _issuer[0] >= 24) else nc.scalar
                _issuer[0] += 1
                eng.dma_start(out=out, in_=in_)

            def load_bias(nm, ext, width):
                if ext is not None:
                    bt = cpool.tile([1, width], bf16, tag=nm)
                    nc.sync.dma_start(out=bt[:], in_=ext[:])
                    bias_tiles[nm] = bt

            def dma_w_dtcol(w_t, ext, dt):
                # one [E, 128] column-slice of a projection weight
                for et in range(ET):
                    dma2(
                        w_t[:, et, dt * 128:(dt + 1) * 128],
                        ext[et * 128:(et + 1) * 128, dt * 128:(dt + 1) * 128],
                    )

            # k/q inputs as single [128, ET, N] tiles so one DMA can span
            # several e-tiles (fewer SP descriptor issues); DRAM side uses a
            # rearranged AP (t p) n -> p t n.
            k_t = wpool.tile([128, ET, N], bf16, tag="k_t")
            q_t = wpool.tile([128, ET, N], bf16, tag="q_t")
            k_tiles = [k_t[:, et, :] for et in range(ET)]
            q_tiles = [q_t[:, et, :] for et in range(ET)]

            def dma_in(dst, ext, n0, n1, et_chunk):
                src = ext.rearrange("(t p) n -> p t n", p=128)
                for e0 in range(0, ET, et_chunk):
                    dma2(
                        dst[:, e0:e0 + et_chunk, n0:n1],
                        src[:, e0:e0 + et_chunk, n0:n1],
                    )

            def dma_w_cols(w_t, ext, d0, d1, et_chunk):
                src = ext.rearrange("(t p) d -> p t d", p=128)
                for e0 in range(0, ET, et_chunk):
                    dma2(
                        w_t[:, e0:e0 + et_chunk, d0:d1],
                        src[:, e0:e0 + et_chunk, d0:d1],
                    )

            # v input: [128, ET, 512] quarter tiles; the 2-slot pool recycles
            # quarter q's tile once V-proj of its n-tiles ran
            v_quarters = {}

            def dma_v_quarter(q, eng=None, ch=1):
                vq = vipool.tile([128, ET, 512], bf16, tag="vin")
                src = vT_ext.rearrange("(t p) n -> p t n", p=128)
                for e0 in range(0, ET, ch):
                    if eng is None:
                        dma2(vq[:, e0:e0 + ch, :],
                             src[:, e0:e0 + ch, q * 512:(q + 1) * 512])
                    else:
                        eng.dma_start(
                            out=vq[:, e0:e0 + ch, :],
                            in_=src[:, e0:e0 + ch, q * 512:(q + 1) * 512],
                        )
                v_quarters[q] = vq

            # DMA issue order is need-order: first-B gate in 128KB chunks
            # (2x queue parallelism; the 8 hw queues serialize per-queue),
            # then the data each later filler group consumes.
            load_bias("bv", bv_ext, S)
            load_bias("bk", bk_ext, S)
            load_bias("bq", bq_ext, S)
            dma_w_cols(wk_t, wk_ext, 0, 128, 4)        # 2 issues (128KB)
            dma_in(k_t, kT_ext, 0, 512, 1)             # 8 (128KB)
            dma_w_cols(wq_t, wq_ext, 0, 128, 4)        # 2
            dma_in(q_t, qT_ext, 0, 512, 1)             # 8
            dma_w_cols(wv_t, wv_ext, 0, S, 1)          # 8 (full wv, 128KB)
            dma_v_quarter(0)                           # 8 (128KB)
            dma_in(k_t, kT_ext, 512, 1024, 2)          # 4
            dma_v_quarter(1)                           # 8
            dma_in(k_t, kT_ext, 1024, 2048, 1)         # 8 (256KB)
            dma_in(q_t, qT_ext, 512, 1024, 2)          # 4
            dma_in(q_t, qT_ext, 1024, 2048, 1)         # 8
            # NOTE: v quarters 2/3 reuse quarter-0/1 slots; their DMAs are
            # emitted inside the (0,0) fillers after the reader vp_groups.
            dma_w_cols(wk_t, wk_ext, 128, 512, 2)      # 4 (dt 1-3)
            dma_w_cols(wq_t, wq_ext, 128, 512, 2)      # 4
            for dt in range(DT):   # wo
                dma2(wo_t[:, dt, :], wo_ext[dt * 128:(dt + 1) * 128, :])
            load_bias("bo", bo_ext, E)

            bv_t = bias_tiles.get("bv")
            bk_t = bias_tiles.get("bk")
            bq_t = bias_tiles.get("bq")
            bo_t = bias_tiles.get("bo")

            # ---- group emitters (each: PSUM group on ps_m + one eviction) --
            def vp_group(nt):
                # Vp[n-tile nt, :] scattered into vpa's 65-stride head layout
                q, r = divmod(nt, 4)
                pt = ps_m.tile([128, 512], f32, tag="d")
                for et in range(ET):
                    nc.tensor.matmul(
                        pt[:, :],
                        v_quarters[q][:, et, r * 128:(r + 1) * 128],
                        wv_t[:, et, :],
                        start=(et == 0),
                        stop=(et == ET - 1 and bv_t is None),
                    )
                if bv_t is not None:
                    nc.tensor.matmul(
                        pt[:, :], ones_bf[0:1, 0:128], bv_t[0:1, :],
                        start=False, stop=True,
                    )
                dst = vpa[:, nt, :].rearrange("p (h c) -> p h c", c=65)[:, :, 0:64]
                src_ = pt[:, :].rearrange("p (h c) -> p h c", c=64)
                nc.vector.tensor_copy(dst, src_)

            def proj_group(in_tiles, w_t, b_t, dest, dt, nb):
                pt = ps_m.tile([128, 512], f32, tag="d")
                n0 = nb * 512
                for et in range(ET):
                    nc.tensor.matmul(
                        pt[:, :],
                        w_t[:, et, dt * 128:(dt + 1) * 128],
                        in_tiles[et][:, n0:n0 + 512],
                        start=(et == 0),
                        stop=(et == ET - 1 and b_t is None),
                    )
                if b_t is not None:
                    nc.tensor.matmul(
                        pt[:, :], b_t[0:1, dt * 128:(dt + 1) * 128],
                        ones_bf[0:1, 0:512], start=False, stop=True,
                    )
                nc.scalar.copy(dest[:, dt, n0:n0 + 512], pt[:, :])

            on_all = [[None] * NBLK for _ in range(DT)]

            def emit_d_group(ibd, etile, pool=None, tag="d"):
                # out-projection for (n-block ibd, e-tile etile)
                pd = (pool if pool is not None else ps_m).tile([128, 512], f32, tag=tag)
                for dt in range(DT):
                    nc.tensor.matmul(
                        pd[:, :],
                        wo_t[:, dt, etile * 128:(etile + 1) * 128],
                        on_all[dt][ibd][:, :],
                        start=(dt == 0),
                        stop=(dt == DT - 1 and bo_t is None),
                    )
                if bo_t is not None:
                    nc.tensor.matmul(
                        pd[:, :],
                        bo_t[0:1, etile * 128:(etile + 1) * 128],
                        ones_bf[0:1, 0:512],
                        start=False, stop=True,
                    )
                ds = dstpool.tile([128, 512], bf16, tag="dst")
                # alternate the eviction engine to balance ACT/DVE load
                if etile % 2 == 0:
                    nc.scalar.copy(ds[:, :], pd[:, :])
                else:
                    nc.vector.tensor_copy(ds[:, :], pd[:, :])
                nc.sync.dma_start(
                    out=out_ext[etile * 128:(etile + 1) * 128,
                                ibd * 512:(ibd + 1) * 512],
                    in_=ds[:, :],
                )

            # ---- static filler schedule: fillers[(hp, ib)][pair] ----------
            def mk(fn, *a):
                return lambda: fn(*a)

            def projK(dt, nb):
                return mk(proj_group, k_tiles, wk_t, bk_t, kpT, dt, nb)

            def projQ(dt, nb):
                return mk(proj_group, q_tiles, wq_t, bq_t, qpT, dt, nb)

            fillers = {(hp, ib): [[] for _ in range(8)] for hp in range(DT) for ib in range(NBLK)}
            f00 = fillers[(0, 0)]
            f00[0] = [mk(vp_group, 2), mk(vp_group, 3), mk(dma_v_quarter, 2, nc.sync, 2)]
            f00[1] = [projK(0, 1)]
            f00[2] = [mk(vp_group, 4), mk(vp_group, 5)]
            f00[3] = [projK(0, 2), mk(vp_group, 6), mk(vp_group, 7), mk(dma_v_quarter, 3, nc.sync, 2)]
            f00[4] = [mk(vp_group, 8), mk(vp_group, 9)]
            f00[5] = [projK(0, 3), mk(vp_group, 10), mk(vp_group, 11), mk(vp_group, 12)]
            f00[6] = [mk(vp_group, 13), mk(vp_group, 14), mk(vp_group, 15)]
            f00[7] = [projQ(0, 1)]
            fillers[(0, 1)][1] = [projQ(0, 2)]
            fillers[(0, 1)][3] = [projK(1, 0)]
            fillers[(0, 1)][5] = [projQ(1, 0)]
            fillers[(0, 2)][1] = [projQ(0, 3)]
            fillers[(0, 2)][3] = [projK(1, 1)]
            fillers[(0, 2)][5] = [projQ(1, 1)]
            fillers[(0, 3)][1] = [projK(1, 2)]
            fillers[(0, 3)][3] = [projK(1, 3)]
            fillers[(0, 3)][5] = [projQ(1, 2)]
            fillers[(0, 3)][7] = [projQ(1, 3)]
            for ib in range(NBLK):
                fillers[(1, ib)][1] = [projK(2, ib)]
                fillers[(1, ib)][5] = [projQ(2, ib)]
            fillers[(2, 0)][1] = [projK(3, 0)]
            fillers[(2, 0)][5] = [projQ(3, 0)]
            fillers[(2, 1)][1] = [projK(3, 1)]
            fillers[(2, 2)][1] = [projK(3, 2)]
            fillers[(2, 3)][1] = [projK(3, 3)]
            fillers[(3, 0)][1] = [projQ(3, 1)]
            fillers[(3, 0)][3] = [projQ(3, 2)]
            fillers[(3, 1)][1] = [projQ(3, 3)]
            # emit_d(ib): ready once normalize(3, ib) stage3 ran (pair 4 of
            # block (3, ib+1)); 3 groups late in (3, ib+1), 5 early in
            # (3, ib+2) (where they double as block-boundary PE absorbers),
            # remainder in the tail.
            tail_d = []
            for ib in range(NBLK):
                slots = [((3, ib + 1), 6), ((3, ib + 1), 7), ((3, ib + 1), 7),
                         ((3, ib + 2), 1), ((3, ib + 2), 1), ((3, ib + 2), 2),
                         ((3, ib + 2), 2), ((3, ib + 2), 3)]
                for gi in range(ET):
                    bkey, pr = slots[gi]
                    if bkey in fillers:
                        fillers[bkey][pr].append(mk(emit_d_group, ib, gi))
                    else:
                        tail_d.append((ib, gi))

            # ---- prelude PE work ------------------------------------------
            proj_group(k_tiles, wk_t, bk_t, kpT, 0, 0)
            proj_group(q_tiles, wq_t, bq_t, qpT, 0, 0)
            vp_group(0)
            vp_group(1)

            # ---- main continuous pipeline over 128 j-tile pairs ------------
            # Normalize is staged across the following block's pairs so no
            # engine queues an instruction whose inputs aren't ready yet
            # (head-of-line blocking on DVE/Pool stalls the exp chain, which
            # stalls the PE and drops it out of its max p-state).
            def norm_stage0(hp, ib, o_a, o_b):
                # evict O+Z to SBUF on ACT (frees the PSUM banks; deps: C15)
                oc_a = nrmpool.tile([65, 512], f32, tag="oc")
                oc_b = nrmpool.tile([65, 512], f32, tag="oc")
                nc.scalar.copy(oc_a[:, :], o_a[:, :])
                nc.scalar.copy(oc_b[:, :], o_b[:, :])
                return oc_a, oc_b

            def norm_stage1(st):
                # Z rows to partition 0 (DMA crosses partitions)
                oc_a, oc_b = st
                zr_a = nrm1pool.tile([1, 512], f32, tag="zr")
                nc.sync.dma_start(out=zr_a[0:1, :], in_=oc_a[64:65, :])
                zr_b = nrm1pool.tile([1, 512], f32, tag="zr")
                nc.sync.dma_start(out=zr_b[0:1, :], in_=oc_b[64:65, :])
                return oc_a, oc_b, zr_a, zr_b

            def norm_stage2(st):
                oc_a, oc_b, zr_a, zr_b = st
                zi_a = nrm1pool.tile([1, 512], f32, tag="zi")
                nc.vector.reciprocal_approx_fast(zi_a[0:1, :], zr_a[0:1, :])
                zi_b = nrm1pool.tile([1, 512], f32, tag="zi")
                nc.vector.reciprocal_approx_fast(zi_b[0:1, :], zr_b[0:1, :])
                return oc_a, oc_b, zi_a, zi_b

            def norm_stage3(st):
                oc_a, oc_b, zi_a, zi_b = st
                zb_a = zbpool.tile([64, 512], f32, tag="zb")
                nc.gpsimd.partition_broadcast(zb_a[:, :], zi_a[0:1, :])
                zb_b = zbpool.tile([64, 512], f32, tag="zb")
                nc.gpsimd.partition_broadcast(zb_b[:, :], zi_b[0:1, :])
                return oc_a, oc_b, zb_a, zb_b

            def norm_stage4(hp, ib, st):
                oc_a, oc_b, zb_a, zb_b = st
                onorm = onpool.tile([128, 512], bf16, tag="onorm")
                nc.gpsimd.tensor_mul(onorm[0:64, :], oc_a[0:64, :], zb_a[:, :])
                tmp_b = tbpool.tile([64, 512], bf16, tag="tmpB")
                nc.gpsimd.tensor_mul(tmp_b[:, :], oc_b[0:64, :], zb_b[:, :])
                # partition shift 0-63 -> 64-127 (DMA crosses partitions)
                nc.sync.dma_start(out=onorm[64:128, :], in_=tmp_b[:, :])
                on_all[hp][ib] = onorm

            def c_mms(hp, jtc, o_a, o_b, esA_t, esB_t):
                nc.tensor.matmul(
                    o_a[:, :],
                    vpa[:, jtc, (2 * hp) * 65:(2 * hp) * 65 + 65],
                    esA_t[:, :],
                    start=(jtc == 0), stop=(jtc == NT - 1),
                )
                nc.tensor.matmul(
                    o_b[:, :],
                    vpa[:, jtc, (2 * hp + 1) * 65:(2 * hp + 1) * 65 + 65],
                    esB_t[:, :].bitcast(bf16),
                    start=(jtc == 0), stop=(jtc == NT - 1),
                )

            pend = []          # [(hp, ib, o_a, o_b, jt, esA, esB)] from prev pair

            class NormFlow:
                # staged normalize: stage1 (Z-row DMAs), stage2 (broadcasts),
                # stage3 (multiplies + partition-shift), run at later pairs
                def __init__(self, hp, ib, st0):
                    self.hp, self.ib, self.st, self.stage = hp, ib, st0, 1

                def step(self):
                    if self.stage == 1:
                        self.st = norm_stage1(self.st)
                    elif self.stage == 2:
                        self.st = norm_stage2(self.st)
                    elif self.stage == 3:
                        self.st = norm_stage3(self.st)
                    elif self.stage == 4:
                        norm_stage4(self.hp, self.ib, self.st)
                    self.stage += 1

            due_stages = []    # [(due_global_pair, NormFlow)] in due order

            def run_due_stages(gp):
                while due_stages and gp >= due_stages[0][0]:
                    due_stages.pop(0)[1].step()

            def drain_pend(gp):
                for (chp, cib, co_a, co_b, jtc, esA_t, esB_t) in pend:
                    c_mms(chp, jtc, co_a, co_b, esA_t, esB_t)
                    if jtc == NT - 1:
                        nf = NormFlow(chp, cib, norm_stage0(chp, cib, co_a, co_b))
                        due_stages.extend([(gp + 1, nf), (gp + 2, nf),
                                           (gp + 3, nf), (gp + 4, nf)])

            blocks = [(hp, ib) for hp in range(DT) for ib in range(NBLK)]
            gp = 0
            for hp, ib in blocks:
                i0 = ib * 512
                o_a = ps_o.tile([65, 512], f32, tag="o")
                o_b = ps_o.tile([65, 512], f32, tag="o")
                for p in range(8):
                    news = []
                    for dj in range(2):
                        jt = 2 * p + dj
                        stA = ps_s.tile([128, 512], f32, tag="s")
                        stB = ps_s.tile([128, 512], f32, tag="s")
                        # head A on PE rows 0-63, head B on rows 64-127
                        nc.tensor.matmul(
                            stA[:, :],
                            kpT[0:64, hp, jt * 128:(jt + 1) * 128],
                            qpT[0:64, hp, i0:i0 + 512],
                            start=True, stop=True,
                        )
                        nc.tensor.matmul(
                            stB[:, :],
                            kpT[64:128, hp, jt * 128:(jt + 1) * 128],
                            qpT[64:128, hp, i0:i0 + 512],
                            start=True, stop=True,
                        )
                        esA_t = esapool.tile([128, 512], bf16, tag="esA")
                        nc.scalar.activation(
                            esA_t[:], stA[:], mybir.ActivationFunctionType.Exp,
                            scale=SCALE,
                        )
                        esB_t = esbpool.tile([128, 512], i16, tag="esB")
                        nc.vector.tensor_scalar(
                            esB_t[:, :], stB[:, :], EXP_A, EXP_B,
                            mybir.AluOpType.mult, mybir.AluOpType.add,
                        )
                        news.append((hp, ib, o_a, o_b, jt, esA_t, esB_t))
                    if p == 1:
                        # fillers first: PE chews them while the previous
                        # block's O banks drain, so C(jt0) never stalls
                        for fn in fillers[(hp, ib)][p]:
                            fn()
                        drain_pend(gp)
                    else:
                        drain_pend(gp)
                        for fn in fillers[(hp, ib)][p]:
                            fn()
                    pend = news
                    run_due_stages(gp)
                    gp += 1
            # tail: drain the last pair, final normalize, final out-projection
            drain_pend(gp)
            pend = []
            while due_stages:
                due_stages.pop(0)[1].step()
            # tail out-projection: the B/C PSUM pool is free now — run the
            # remaining groups through its 5 slots so evictions pipeline
            for ibd, et in tail_d:
                emit_d_group(ibd, et, pool=ps_s, tag="s")

    nc.compile()
    return nc


def _bf16c(a):
    return np.ascontiguousarray(a, dtype=np.float32).astype(BF16NP)


def kernel(q, k, v, Wq, bq, Wk, bk, Wv, bv, Wo, bo, trace=False):
    global last_exec_time_ns, last_results
    q = np.asarray(q, dtype=np.float32)
    k = np.asarray(k, dtype=np.float32)
    v = np.asarray(v, dtype=np.float32)
    Wq, Wk, Wv, Wo = (np.asarray(x, dtype=np.float32) for x in (Wq, Wk, Wv, Wo))
    bq, bk, bv, bo = (np.asarray(x, dtype=np.float32) for x in (bq, bk, bv, bo))

    has_bq, has_bk, has_bv, has_bo = (bool(np.any(x)) for x in (bq, bk, bv, bo))

    _install_ntff_shim()
    nc = _build(has_bq, has_bk, has_bv, has_bo)

    in_maps = []
    for c in range(8):
        b, g = divmod(c, 2)
        sl = slice(g * S, (g + 1) * S)
        m = {
            "qT": _bf16c(q[b].T),
            "kT": _bf16c(k[b].T),
            "vT": _bf16c(v[b].T),
            "wq": _bf16c(Wq[:, sl]),
            "wk": _bf16c(Wk[:, sl]),
            "wv": _bf16c(Wv[:, sl]),
            "wo": _bf16c(Wo[sl, :]),
        }
        if has_bq:
            m["bq"] = _bf16c(bq[sl].reshape(1, S))
        if has_bk:
            m["bk"] = _bf16c(bk[sl].reshape(1, S))
        if has_bv:
            m["bv"] = _bf16c(bv[sl].reshape(1, S))
        if has_bo:
            m["bo"] = _bf16c((bo if g == 0 else np.zeros_like(bo)).reshape(1, E))
        in_maps.append(m)

    res = run_bass_kernel_spmd(nc, in_maps, core_ids=list(range(8)), trace=trace)
    last_results = res
    last_exec_time_ns = res.exec_time_ns

    out = np.empty((B, N, E), dtype=np.float32)
    for b in range(B):
        out[b] = (res.results[2 * b]["out"].astype(np.float32)
                  + res.results[2 * b + 1]["out"].astype(np.float32)).T
    return out


# revision 37
# speedup vs baseline: 1.6427x; 1.6427x over previous
"""Distributed multi-head attention kernel for 8 Trainium2 NeuronCores.

Problem: B=4, N=2048, E=1024, H=16 heads (head_dim 64), QKV + out projections.
Sharding: core c handles batch b=c//2 and head-group g=c%2 (8 heads = D-slice
of 512). QKV projections are column-sharded, the out projection is row-sharded;
the two partial outputs per batch are summed on the host during unshard.

Per-core dataflow (all matmuls bf16 with fp32 PSUM accumulation):
  A) QpT/KpT [512, 2048] and Vp [2048, 512] projections. Host pre-transposes
     q/k/v to [E, N] so the contraction dim lands on SBUF partitions.
  B) S^T[j, i] = Kp_h^T.T @ Qp_h^T per head. K=64, so head pairs are packed
     onto PE row-groups 0-63 / 64-127 (partition-base-derived tile_position).
     Each head's S^T j-tile goes to its own single-bank [128, 512] PSUM tile.
  C) exp split across engines to break the single-ACT bottleneck: head A's
     tile exp's on the scalar engine (LUT exp fused into the PSUM eviction);
     head B's tile exp's on the DVE as a Schraudolph bf16-bit trick
     (round(s*EXP_A + EXP_B) -> int16, bitcast bf16). The Schraudolph noise
     (~1.9% rms on half the P mass) keeps end-to-end rel err ~1.4e-2 < 2e-2.
  D) O^T_aug[65, i] accumulates Vp_aug^T @ expS^T over j-tiles, where Vp is
     augmented with a ones column so row 64 of the product is the softmax
     denominator Z. Normalize: evict O_aug (DVE), reciprocal (DVE),
     partition-broadcast + multiplies on Pool (SBUF-only engine), partition
     shift 0-63 -> 64-127 via DMA; out-project, store out^T bf16 partials.
  The phases stream as one continuous pipeline: B runs a pair of j-tiles
  ahead of C, V/QK projections and the out-projection are spread as PE
  filler so the array never waits on the exp chain.
"""

import sys

import numpy as np


def _ensure_paths():
    try:
        import concourse.bass  # noqa: F401
    except ImportError:
        for p in ("/opt/trn_rl_repo",):
            if p not in sys.path:
                sys.path.insert(0, p)
        import concourse.bass  # noqa: F401


_ensure_paths()

import ml_dtypes  # noqa: E402
import concourse.bass as bass  # noqa: E402
import concourse.bacc as bacc  # noqa: E402
import concourse.mybir as mybir  # noqa: E402
import concourse.tile as tile  # noqa: E402
from concourse.bass_utils import run_bass_kernel_spmd  # noqa: E402

BF16NP = ml_dtypes.bfloat16

B, N, E = 4, 2048, 1024
H, HD = 16, 64
G = 2                 # head-group (tensor-parallel) factor
S = E // G            # 512: per-core slice of the internal dim
HPC = H // G          # 8 heads per core
ET = E // 128         # 8 contraction tiles for the projections
DT = S // 128         # 4 d-tiles per core (= head pairs)
NT = N // 128         # 16 n-tiles
NBLK = N // 512       # 4 n/i blocks
SCALE = 1.0 / float(np.sqrt(HD))
# Schraudolph exp in bf16-bit space: bf16_bits(exp(s*SCALE)) ~= s*EXP_A + EXP_B
# (DVE f32->int16 convert rounds to nearest; B tuned for ~zero mean bias)
EXP_A = 128.0 * 1.4426950408889634 * SCALE
EXP_B = 127.0 * 128.0 - 7.2

last_exec_time_ns = None
last_results = None


def _install_ntff_shim():
    """Register the axon NTFF profile hook bass_utils wants under trace=True."""
    import types

    if "antenv.axon_hooks" in sys.modules:
        return
    mod = types.ModuleType("antenv.axon_hooks")
    _h = [None]
    mod.set_axon_ntff_profile_hook = lambda h: _h.__setitem__(0, h)
    mod.get_axon_ntff_profile_hook = lambda: _h[0]
    try:
        import antenv

        sys.modules["antenv.axon_hooks"] = mod
        antenv.axon_hooks = mod
        from trn_agent_boot.trn_boot import _ntff_profile_via_ctypes

        hook = _ntff_profile_via_ctypes("/opt/axon/libaxon_pjrt.so")
        if hook is not None:
            mod.set_axon_ntff_profile_hook(hook)
    except Exception:
        pass


def _build(has_bq, has_bk, has_bv, has_bo):
    f32 = mybir.dt.float32
    bf16 = mybir.dt.bfloat16
    i16 = mybir.dt.int16
    PSUM = bass.MemorySpace.PSUM

    nc = bacc.Bacc("TRN2", target_bir_lowering=False, debug=False)

    qT_ext = nc.declare_dram_parameter("qT", [E, N], bf16, isOutput=False)
    kT_ext = nc.declare_dram_parameter("kT", [E, N], bf16, isOutput=False)
    vT_ext = nc.declare_dram_parameter("vT", [E, N], bf16, isOutput=False)
    wq_ext = nc.declare_dram_parameter("wq", [E, S], bf16, isOutput=False)
    wk_ext = nc.declare_dram_parameter("wk", [E, S], bf16, isOutput=False)
    wv_ext = nc.declare_dram_parameter("wv", [E, S], bf16, isOutput=False)
    wo_ext = nc.declare_dram_parameter("wo", [S, E], bf16, isOutput=False)
    bq_ext = nc.declare_dram_parameter("bq", [1, S], bf16, isOutput=False) if has_bq else None
    bk_ext = nc.declare_dram_parameter("bk", [1, S], bf16, isOutput=False) if has_bk else None
    bv_ext = nc.declare_dram_parameter("bv", [1, S], bf16, isOutput=False) if has_bv else None
    bo_ext = nc.declare_dram_parameter("bo", [1, E], bf16, isOutput=False) if has_bo else None
    out_ext = nc.declare_dram_parameter("out", [E, N], bf16, isOutput=True)

    with tile.TileContext(nc) as tc:
        with (
            tc.tile_pool(name="const", bufs=1) as cpool,
            tc.tile_pool(name="w", bufs=1) as wpool,
            tc.tile_pool(name="vin", bufs=2) as vipool,
            tc.tile_pool(name="proj", bufs=1) as ppool,
            tc.tile_pool(name="esA", bufs=3) as esapool,
            tc.tile_pool(name="esB", bufs=3) as esbpool,
            tc.tile_pool(name="on", bufs=13) as onpool,
            tc.tile_pool(name="nrm", bufs=4) as nrmpool,
            tc.tile_pool(name="nrm1", bufs=2) as nrm1pool,
            tc.tile_pool(name="zb", bufs=2) as zbpool,
            tc.tile_pool(name="tmpb", bufs=2) as tbpool,
            tc.tile_pool(name="dst", bufs=4) as dstpool,
            tc.tile_pool(name="ps_s", bufs=5, space=PSUM) as ps_s,
            tc.tile_pool(name="ps_o", bufs=2, space=PSUM) as ps_o,
            tc.tile_pool(name="ps_m", bufs=1, space=PSUM) as ps_m,
        ):
            # ---- constants -------------------------------------------------
            ones_bf = cpool.tile([1, 512], bf16, tag="ones_bf")
            nc.gpsimd.memset(ones_bf[:], 1.0)

            # ---- persistent activations -----------------------------------
            qpT = ppool.tile([128, DT, N], bf16, tag="qpT")   # [d, n], d-tiles = head pairs
            kpT = ppool.tile([128, DT, N], bf16, tag="kpT")
            vpa = ppool.tile([128, NT, HPC * 65], bf16, tag="vpa")  # per head: 64 V cols + ones col
            nc.gpsimd.memset(vpa[:], 1.0)  # pre-set so the ones columns survive the V copies

            # ---- weights / inputs: declared here, DMA'd in priority order --
            wq_t = wpool.tile([128, ET, S], bf16, tag="wq")
            wk_t = wpool.tile([128, ET, S], bf16, tag="wk")
            wv_t = wpool.tile([128, ET, S], bf16, tag="wv")
            wo_t = wpool.tile([128, DT, E], bf16, tag="wo")

            bias_tiles = {}

            # SP issues each DMA descriptor serially (~650ns); the prelude
            # loads alternate between the two HWDGE issuers (SP + ACT) to
            # halve the serial issue time while ACT is still idle.
            _issuer = [0]

            def dma2(out, in_):
                # alternate issuers for the first 24 loads only: past that the
                # ACT queue must stay clear for projection evictions / exps
                # (a queued DMA's queue-credit wait would block them)
                eng = nc.sync if (_issuer[0] % 2 == 0 or _issuer[0] >= 24) else nc.scalar
                _issuer[0] += 1
                eng.dma_start(out=out, in_=in_)

            def load_bias(nm, ext, width):
                if ext is not None:
                    bt = cpool.tile([1, width], bf16, tag=nm)
                    nc.sync.dma_start(out=bt[:], in_=ext[:])
                    bias_tiles[nm] = bt

            def dma_w_dtcol(w_t, ext, dt):
                # one [E, 128] column-slice of a projection weight
                for et in range(ET):
                    dma2(
                        w_t[:, et, dt * 128:(dt + 1) * 128],
                        ext[et * 128:(et + 1) * 128, dt * 128:(dt + 1) * 128],
                    )

            # k/q inputs as single [128, ET, N] tiles so one DMA can span
            # several e-tiles (fewer SP descriptor issues); DRAM side uses a
            # rearranged AP (t p) n -> p t n.
            k_t = wpool.tile([128, ET, N], bf16, tag="k_t")
            q_t = wpool.tile([128, ET, N], bf16, tag="q_t")
            k_tiles = [k_t[:, et, :] for et in range(ET)]
            q_tiles = [q_t[:, et, :] for et in range(ET)]

            def dma_in(dst, ext, n0, n1, et_chunk):
                src = ext.rearrange("(t p) n -> p t n", p=128)
                for e0 in range(0, ET, et_chunk):
                    dma2(
                        dst[:, e0:e0 + et_chunk, n0:n1],
                        src[:, e0:e0 + et_chunk, n0:n1],
                    )

            def dma_w_cols(w_t, ext, d0, d1, et_chunk):
                src = ext.rearrange("(t p) d -> p t d", p=128)
                for e0 in range(0, ET, et_chunk):
                    dma2(
                        w_t[:, e0:e0 + et_chunk, d0:d1],
                        src[:, e0:e0 + et_chunk, d0:d1],
                    )

            # v input: [128, ET, 512] quarter tiles; the 2-slot pool recycles
            # quarter q's tile once V-proj of its n-tiles ran
            v_quarters = {}

            def dma_v_quarter(q, eng=None, ch=1):
                vq = vipool.tile([128, ET, 512], bf16, tag="vin")
                src = vT_ext.rearrange("(t p) n -> p t n", p=128)
                for e0 in range(0, ET, ch):
                    if eng is None:
                        dma2(vq[:, e0:e0 + ch, :],
                             src[:, e0:e0 + ch, q * 512:(q + 1) * 512])
                    else:
                        eng.dma_start(
                            out=vq[:, e0:e0 + ch, :],
                            in_=src[:, e0:e0 + ch, q * 512:(q + 1) * 512],
                        )
                v_quarters[q] = vq

            # DMA issue order is need-order: first-B gate in 128KB chunks
            # (2x queue parallelism; the 8 hw queues serialize per-queue),
            # then the data each later filler group consumes.
            load_bias("bv", bv_ext, S)
            load_bias("bk", bk_ext, S)
            load_bias("bq", bq_ext, S)
            dma_w_cols(wk_t, wk_ext, 0, 128, 4)        # 2 issues (128KB)
            dma_in(k_t, kT_ext, 0, 512, 1)             # 8 (128KB)
            dma_w_cols(wq_t, wq_ext, 0, 128, 4)        # 2
            dma_in(q_t, qT_ext, 0, 512, 1)             # 8
            dma_w_cols(wv_t, wv_ext, 0, S, 1)          # 8 (full wv, 128KB)
            dma_v_quarter(0)                           # 8 (128KB)
            dma_in(k_t, kT_ext, 512, 1024, 2)          # 4
            dma_v_quarter(1)                           # 8
            dma_in(k_t, kT_ext, 1024, 2048, 1)         # 8 (256KB)
            dma_in(q_t, qT_ext, 512, 1024, 2)          # 4
            dma_in(q_t, qT_ext, 1024, 2048, 1)         # 8
            # NOTE: v quarters 2/3 reuse quarter-0/1 slots; their DMAs are
            # emitted inside the (0,0) fillers after the reader vp_groups.
            dma_w_cols(wk_t, wk_ext, 128, 512, 2)      # 4 (dt 1-3)
            dma_w_cols(wq_t, wq_ext, 128, 512, 2)      # 4
            for dt in range(DT):   # wo
                dma2(wo_t[:, dt, :], wo_ext[dt * 128:(dt + 1) * 128, :])
            load_bias("bo", bo_ext, E)

            bv_t = bias_tiles.get("bv")
            bk_t = bias_tiles.get("bk")
            bq_t = bias_tiles.get("bq")
            bo_t = bias_tiles.get("bo")

            # ---- group emitters (each: PSUM group on ps_m + one eviction) --
            def vp_group(nt):
                # Vp[n-tile nt, :] scattered into vpa's 65-stride head layout
                q, r = divmod(nt, 4)
                pt = ps_m.tile([128, 512], f32, tag="d")
                for et in range(ET):
                    nc.tensor.matmul(
                        pt[:, :],
                        v_quarters[q][:, et, r * 128:(r + 1) * 128],
                        wv_t[:, et, :],
                        start=(et == 0),
                        stop=(et == ET - 1 and bv_t is None),
                    )
                if bv_t is not None:
                    nc.tensor.matmul(
                        pt[:, :], ones_bf[0:1, 0:128], bv_t[0:1, :],
                        start=False, stop=True,
                    )
                dst = vpa[:, nt, :].rearrange("p (h c) -> p h c", c=65)[:, :, 0:64]
                src_ = pt[:, :].rearrange("p (h c) -> p h c", c=64)
                nc.vector.tensor_copy(dst, src_)

            def proj_group(in_tiles, w_t, b_t, dest, dt, nb):
                pt = ps_m.tile([128, 512], f32, tag="d")
                n0 = nb * 512
                for et in range(ET):
                    nc.tensor.matmul(
                        pt[:, :],
                        w_t[:, et, dt * 128:(dt + 1) * 128],
                        in_tiles[et][:, n0:n0 + 512],
                        start=(et == 0),
                        stop=(et == ET - 1 and b_t is None),
                    )
                if b_t is not None:
                    nc.tensor.matmul(
                        pt[:, :], b_t[0:1, dt * 128:(dt + 1) * 128],
                        ones_bf[0:1, 0:512], start=False, stop=True,
                    )
                nc.scalar.copy(dest[:, dt, n0:n0 + 512], pt[:, :])

            on_all = [[None] * NBLK for _ in range(DT)]

            def emit_d_group(ibd, etile, pool=None, tag="d"):
                # out-projection for (n-block ibd, e-tile etile)
                pd = (pool if pool is not None else ps_m).tile([128, 512], f32, tag=tag)
                for dt in range(DT):
                    nc.tensor.matmul(
                        pd[:, :],
                        wo_t[:, dt, etile * 128:(etile + 1) * 128],
                        on_all[dt][ibd][:, :],
                        start=(dt == 0),
                        stop=(dt == DT - 1 and bo_t is None),
                    )
                if bo_t is not None:
                    nc.tensor.matmul(
                        pd[:, :],
                        bo_t[0:1, etile * 128:(etile + 1) * 128],
                        ones_bf[0:1, 0:512],
                        start=False, stop=True,
                    )
                ds = dstpool.tile([128, 512], bf16, tag="dst")
                # alternate the eviction engine to balance ACT/DVE load
                if etile % 2 == 0:
                    nc.scalar.copy(ds[:, :], pd[:, :])
                else:
                    nc.vector.tensor_copy(ds[:, :], pd[:, :])
                nc.sync.dma_start(
                    out=out_ext[etile * 128:(etile + 1) * 128,
                                ibd * 512:(ibd + 1) * 512],
                    in_=ds[:, :],
                )

            # ---- static filler schedule: fillers[(hp, ib)][pair] ----------
            def mk(fn, *a):
                return lambda: fn(*a)

            def projK(dt, nb):
                return mk(proj_group, k_tiles, wk_t, bk_t, kpT, dt, nb)

            def projQ(dt, nb):
                return mk(proj_group, q_tiles, wq_t, bq_t, qpT, dt, nb)

            fillers = {(hp, ib): [[] for _ in range(8)] for hp in range(DT) for ib in range(NBLK)}
            f00 = fillers[(0, 0)]
            f00[0] = [mk(vp_group, 2), mk(vp_group, 3), mk(dma_v_quarter, 2, nc.sync, 2)]
            f00[1] = [projK(0, 1)]
            f00[2] = [mk(vp_group, 4), mk(vp_group, 5)]
            f00[3] = [projK(0, 2), mk(vp_group, 6), mk(vp_group, 7), mk(dma_v_quarter, 3, nc.sync, 2)]
            f00[4] = [mk(vp_group, 8), mk(vp_group, 9)]
            f00[5] = [projK(0, 3), mk(vp_group, 10), mk(vp_group, 11), mk(vp_group, 12)]
            f00[6] = [mk(vp_group, 13), mk(vp_group, 14), mk(vp_group, 15)]
            f00[7] = [projQ(0, 1)]
            fillers[(0, 1)][1] = [projQ(0, 2)]
            fillers[(0, 1)][3] = [projK(1, 0)]
            fillers[(0, 1)][5] = [projQ(1, 0)]
            fillers[(0, 2)][1] = [projQ(0, 3)]
            fillers[(0, 2)][3] = [projK(1, 1)]
            fillers[(0, 2)][5] = [projQ(1, 1)]
            fillers[(0, 3)][1] = [projK(1, 2)]
            fillers[(0, 3)][3] = [projK(1, 3)]
            fillers[(0, 3)][5] = [projQ(1, 2)]
            fillers[(0, 3)][7] = [projQ(1, 3)]
            for ib in range(NBLK):
                fillers[(1, ib)][1] = [projK(2, ib)]
                fillers[(1, ib)][5] = [projQ(2, ib)]
            fillers[(2, 0)][1] = [projK(3, 0)]
            fillers[(2, 0)][5] = [projQ(3, 0)]
            fillers[(2, 1)][1] = [projK(3, 1)]
            fillers[(2, 2)][1] = [projK(3, 2)]
            fillers[(2, 3)][1] = [projK(3, 3)]
            fillers[(3, 0)][1] = [projQ(3, 1)]
            fillers[(3, 0)][3] = [projQ(3, 2)]
            fillers[(3, 1)][1] = [projQ(3, 3)]
            # emit_d(ib): ready once normalize(3, ib) stage3 ran (pair 4 of
            # block (3, ib+1)); 3 groups late in (3, ib+1), 5 early in
            # (3, ib+2) (where they double as block-boundary PE absorbers),
            # remainder in the tail.
            tail_d = []
            for ib in range(NBLK):
                slots = [((3, ib + 1), 6), ((3, ib + 1), 7), ((3, ib + 1), 7),
                         ((3, ib + 2), 1), ((3, ib + 2), 1), ((3, ib + 2), 2),
                         ((3, ib + 2), 2), ((3, ib + 2), 3)]
                for gi in range(ET):
                    bkey, pr = slots[gi]
                    if bkey in fillers:
                        fillers[bkey][pr].append(mk(emit_d_group, ib, gi))
                    else:
                        tail_d.append((ib, gi))

            # ---- prelude PE work ------------------------------------------
            proj_group(k_tiles, wk_t, bk_t, kpT, 0, 0)
            proj_group(q_tiles, wq_t, bq_t, qpT, 0, 0)
            vp_group(0)
            vp_group(1)

            # ---- main continuous pipeline over 128 j-tile pairs ------------
            # Normalize is staged across the following block's pairs so no
            # engine queues an instruction whose inputs aren't ready yet
            # (head-of-line blocking on DVE/Pool stalls the exp chain, which
            # stalls the PE and drops it out of its max p-state).
            def norm_stage0(hp, ib, o_a, o_b):
                # evict O+Z to SBUF on ACT (frees the PSUM banks; deps: C15)
                oc_a = nrmpool.tile([65, 512], f32, tag="oc")
                oc_b = nrmpool.tile([65, 512], f32, tag="oc")
                nc.scalar.copy(oc_a[:, :], o_a[:, :])
                nc.scalar.copy(oc_b[:, :], o_b[:, :])
                return oc_a, oc_b

            def norm_stage1(st):
                # Z rows to partition 0 (DMA crosses partitions)
                oc_a, oc_b = st
                zr_a = nrm1pool.tile([1, 512], f32, tag="zr")
                nc.sync.dma_start(out=zr_a[0:1, :], in_=oc_a[64:65, :])
                zr_b = nrm1pool.tile([1, 512], f32, tag="zr")
                nc.sync.dma_start(out=zr_b[0:1, :], in_=oc_b[64:65, :])
                return oc_a, oc_b, zr_a, zr_b

            def norm_stage2(st):
                oc_a, oc_b, zr_a, zr_b = st
                zi_a = nrm1pool.tile([1, 512], f32, tag="zi")
                nc.vector.reciprocal_approx_fast(zi_a[0:1, :], zr_a[0:1, :])
                zi_b = nrm1pool.tile([1, 512], f32, tag="zi")
                nc.vector.reciprocal_approx_fast(zi_b[0:1, :], zr_b[0:1, :])
                return oc_a, oc_b, zi_a, zi_b

            def norm_stage3(st):
                oc_a, oc_b, zi_a, zi_b = st
                zb_a = zbpool.tile([64, 512], f32, tag="zb")
                nc.gpsimd.partition_broadcast(zb_a[:, :], zi_a[0:1, :])
                zb_b = zbpool.tile([64, 512], f32, tag="zb")
                nc.gpsimd.partition_broadcast(zb_b[:, :], zi_b[0:1, :])
                return oc_a, oc_b, zb_a, zb_b

            def norm_stage4(hp, ib, st):
                oc_a, oc_b, zb_a, zb_b = st
                onorm = onpool.tile([128, 512], bf16, tag="onorm")
                nc.vector.tensor_mul(onorm[0:64, :], oc_a[0:64, :], zb_a[:, :])
                tmp_b = tbpool.tile([64, 512], bf16, tag="tmpB")
                nc.vector.tensor_mul(tmp_b[:, :], oc_b[0:64, :], zb_b[:, :])
                # partition shift 0-63 -> 64-127 (DMA crosses partitions)
                nc.sync.dma_start(out=onorm[64:128, :], in_=tmp_b[:, :])
                on_all[hp][ib] = onorm

            def c_mms(hp, jtc, o_a, o_b, esA_t, esB_t):
                nc.tensor.matmul(
                    o_a[:, :],
                    vpa[:, jtc, (2 * hp) * 65:(2 * hp) * 65 + 65],
                    esA_t[:, :],
                    start=(jtc == 0), stop=(jtc == NT - 1),
                )
                nc.tensor.matmul(
                    o_b[:, :],
                    vpa[:, jtc, (2 * hp + 1) * 65:(2 * hp + 1) * 65 + 65],
                    esB_t[:, :].bitcast(bf16),
                    start=(jtc == 0), stop=(jtc == NT - 1),
                )

            pend = []          # [(hp, ib, o_a, o_b, jt, esA, esB)] from prev pair

            class NormFlow:
                # staged normalize: stage1 (Z-row DMAs), stage2 (broadcasts),
                # stage3 (multiplies + partition-shift), run at later pairs
                def __init__(self, hp, ib, st0):
                    self.hp, self.ib, self.st, self.stage = hp, ib, st0, 1

                def step(self):
                    if self.stage == 1:
                        self.st = norm_stage1(self.st)
                    elif self.stage == 2:
                        self.st = norm_stage2(self.st)
                    elif self.stage == 3:
                        self.st = norm_stage3(self.st)
                    elif self.stage == 4:
                        norm_stage4(self.hp, self.ib, self.st)
                    self.stage += 1

            due_stages = []    # [(due_global_pair, NormFlow)] in due order

            def run_due_stages(gp):
                while due_stages and gp >= due_stages[0][0]:
                    due_stages.pop(0)[1].step()

            def drain_pend(gp):
                for (chp, cib, co_a, co_b, jtc, esA_t, esB_t) in pend:
                    c_mms(chp, jtc, co_a, co_b, esA_t, esB_t)
                    if jtc == NT - 1:
                        nf = NormFlow(chp, cib, norm_stage0(chp, cib, co_a, co_b))
                        due_stages.extend([(gp + 1, nf), (gp + 2, nf),
                                           (gp + 3, nf), (gp + 4, nf)])

            blocks = [(hp, ib) for hp in range(DT) for ib in range(NBLK)]
            gp = 0
            for hp, ib in blocks:
                i0 = ib * 512
                o_a = ps_o.tile([65, 512], f32, tag="o")
                o_b = ps_o.tile([65, 512], f32, tag="o")
                for p in range(8):
                    news = []
                    for dj in range(2):
                        jt = 2 * p + dj
                        stA = ps_s.tile([128, 512], f32, tag="s")
                        stB = ps_s.tile([128, 512], f32, tag="s")
                        # head A on PE rows 0-63, head B on rows 64-127
                        nc.tensor.matmul(
                            stA[:, :],
                            kpT[0:64, hp, jt * 128:(jt + 1) * 128],
                            qpT[0:64, hp, i0:i0 + 512],
                            start=True, stop=True,
                        )
                        nc.tensor.matmul(
                            stB[:, :],
                            kpT[64:128, hp, jt * 128:(jt + 1) * 128],
                            qpT[64:128, hp, i0:i0 + 512],
                            start=True, stop=True,
                        )
                        esA_t = esapool.tile([128, 512], bf16, tag="esA")
                        nc.scalar.activation(
                            esA_t[:], stA[:], mybir.ActivationFunctionType.Exp,
                            scale=SCALE,
                        )
                        esB_t = esbpool.tile([128, 512], i16, tag="esB")
                        nc.vector.tensor_scalar(
                            esB_t[:, :], stB[:, :], EXP_A, EXP_B,
                            mybir.AluOpType.mult, mybir.AluOpType.add,
                        )
                        news.append((hp, ib, o_a, o_b, jt, esA_t, esB_t))
                    if p == 1:
                        # fillers first: PE chews them while the previous
                        # block's O banks drain, so C(jt0) never stalls
                        for fn in fillers[(hp, ib)][p]:
                            fn()
                        drain_pend(gp)
                    else:
                        drain_pend(gp)
                        for fn in fillers[(hp, ib)][p]:
                            fn()
                    pend = news
                    run_due_stages(gp)
                    gp += 1
            # tail: drain the last pair, final normalize, final out-projection
            drain_pend(gp)
            pend = []
            while due_stages:
                due_stages.pop(0)[1].step()
            # tail out-projection: the B/C PSUM pool is free now — run the
            # remaining groups through its 5 slots so evictions pipeline
            for ibd, et in tail_d:
                emit_d_group(ibd, et, pool=ps_s, tag="s")

    nc.compile()
    return nc


def _bf16c(a):
    return np.ascontiguousarray(a, dtype=np.float32).astype(BF16NP)


def kernel(q, k, v, Wq, bq, Wk, bk, Wv, bv, Wo, bo, trace=False):
    global last_exec_time_ns, last_results
    q = np.asarray(q, dtype=np.float32)
    k = np.asarray(k, dtype=np.float32)
    v = np.asarray(v, dtype=np.float32)
    Wq, Wk, Wv, Wo = (np.asarray(x, dtype=np.float32) for x in (Wq, Wk, Wv, Wo))
    bq, bk, bv, bo = (np.asarray(x, dtype=np.float32) for x in (bq, bk, bv, bo))

    has_bq, has_bk, has_bv, has_bo = (bool(np.any(x)) for x in (bq, bk, bv, bo))

    _install_ntff_shim()
    nc = _build(has_bq, has_bk, has_bv, has_bo)

    in_maps = []
    for c in range(8):
        b, g = divmod(c, 2)
        sl = slice(g * S, (g + 1) * S)
        m = {
            "qT": _bf16c(q[b].T),
            "kT": _bf16c(k[b].T),
            "vT": _bf16c(v[b].T),
            "wq": _bf16c(Wq[:, sl]),
            "wk": _bf16c(Wk[:, sl]),
            "wv": _bf16c(Wv[:, sl]),
            "wo": _bf16c(Wo[sl, :]),
        }
        if has_bq:
            m["bq"] = _bf16c(bq[sl].reshape(1, S))
        if has_bk:
            m["bk"] = _bf16c(bk[sl].reshape(1, S))
        if has_bv:
            m["bv"] = _bf16c(bv[sl].reshape(1, S))
        if has_bo:
            m["bo"] = _bf16c((bo if g == 0 else np.zeros_like(bo)).reshape(1, E))
        in_maps.append(m)

    res = run_bass_kernel_spmd(nc, in_maps, core_ids=list(range(8)), trace=trace)
    last_results = res
    last_exec_time_ns = res.exec_time_ns

    out = np.empty((B, N, E), dtype=np.float32)
    for b in range(B):
        out[b] = (res.results[2 * b]["out"].astype(np.float32)
                  + res.results[2 * b + 1]["out"].astype(np.float32)).T
    return out
